# revision 1
# baseline (speedup 1.0000x reference)
"""Multi-head attention (B=2, S=2048, D=1024, H=16) on 8 trn2 NeuronCores.

Sharding: data-parallel over batch (2) x tensor-parallel over head-groups (4).
Core c handles batch c//4, heads [4*(c%4), 4*(c%4)+4).  Each core computes
Q/K/V projections for its 4 heads, attention (no mask - the reference's
causal mask is a no-op), and a partial out-projection against its slice of
Wo.  The 4 partial outputs per batch are summed on the host (+bias), which
replaces the all-reduce.

Device pipeline (per core):
  - Host pre-transposes and pre-tiles x and the weights into the exact SBUF
    layouts, so every input lands with one large contiguous DMA (no
    on-device transposes, minimal DMA count).
  - x.T is streamed in 4 s-chunks; Q^T/K^T/V projections for a chunk start
    as soon as its DMA lands, and the attention k-tile loop starts once the
    first chunk's projections are done - DMA, projections and attention all
    overlap.
  - Scores are computed transposed (ST[k,q]) with two heads row-packed into
    the PE array (K=64 each, tile_position rows 0/64); exp(s/8) for both
    heads runs as one ACT instruction straight out of PSUM.  The softmax
    denominator is obtained by appending a ones-column to V (lhsT =
    [V_h | 1]): row 64 of the ctx accumulation is the denominator.
  - Normalization: reciprocal of the denom row, partition-broadcast with a
    K=1 matmul against ones, then a DVE multiply.  Odd heads are staged and
    DMA-shifted from partitions 0:64 to 64:128 so the out-projection can
    run K=128 matmuls on head pairs.
  - Matmuls run in float32r (fp32 bits, reduced-precision PE mode, 4x
    faster than strict fp32; measured output error ~4e-4 relative).
"""

import numpy as np

import concourse.mybir as mybir
from concourse import bacc
from concourse.tile import TileContext
from concourse.bass_utils import run_bass_kernel_spmd

# problem constants (hardcoded; kernel.py must be self-contained)
B, S, D, H, HD = 2, 2048, 1024, 16, 64
GROUPS = 4                 # head-groups (tensor-parallel)
HG = H // GROUPS           # heads per core = 4
DV = HG * HD               # per-core qkv width = 256
P = 128
DC = D // P                # 8 contraction chunks
ST = S // P                # 16 s/k tiles
NQ = 512                   # moving free dim / q-chunk
QC = S // NQ               # 4 q-chunks
NCORES = 8

f32 = mybir.dt.float32
f32r = mybir.dt.float32r
EXP = mybir.ActivationFunctionType.Exp

_CACHE = {}


def _build(reps=1, mode="full"):
    """Build the per-core program. reps>1 repeats the whole computation
    inside a hardware loop; mode in ("full", "noout", "proj", "dma")
    ablates phases — both only used for timing attribution."""
    nc = bacc.Bacc(None, target_bir_lowering=False, debug=False)

    # All inputs host-packed to match SBUF layouts (one contiguous DMA each)
    xt_d = nc.dram_tensor("xt", [QC, P, DC, NQ], f32r, kind="ExternalInput")
    wqt_d = nc.dram_tensor("wqt", [P, DC, DV], f32r, kind="ExternalInput")
    wkt_d = nc.dram_tensor("wkt", [P, DC, DV], f32r, kind="ExternalInput")
    wvt_d = nc.dram_tensor("wvt", [P, DC, DV], f32r, kind="ExternalInput")
    wot_d = nc.dram_tensor("wot", [P, 2, D], f32r, kind="ExternalInput")
    out_d = nc.dram_tensor("out", [S, D], f32, kind="ExternalOutput")

    from contextlib import ExitStack
    with TileContext(nc) as tc, ExitStack() as rep_ctx:
        if reps > 1:
            rep_ctx.enter_context(tc.For_i(0, reps, 1))
        with (
            tc.tile_pool(name="persist", bufs=1) as pp,
            tc.tile_pool(name="xtp", bufs=2) as xtp,
            tc.tile_pool(name="pt", bufs=8) as ptp,
            tc.tile_pool(name="osb", bufs=3) as osb,
            tc.tile_pool(name="wkp", bufs=2) as wkp,
            tc.tile_pool(name="ps2", bufs=2, space="PSUM") as ps2,
            tc.tile_pool(name="stp", bufs=2, space="PSUM") as stp,
            tc.tile_pool(name="ctp", bufs=2, space="PSUM") as ctp,
        ):
            # QT/KT tile t holds heads 2t (partitions 0:64) and 2t+1 (64:128)
            qt_sb = pp.tile([P, 2, S], f32r)
            kt_sb = pp.tile([P, 2, S], f32r)
            # V per (k-tile, head) with a ones column appended ([V | 1])
            vp_sb = pp.tile([P, ST, HG, HD + 1], f32r)
            # normalized ctx^T paired like QT/KT: tile t = heads 2t, 2t+1
            ctn_sb = pp.tile([P, 2, S], f32r)
            wot_sb = pp.tile([P, 2, D], f32r)
            ones_sb = pp.tile([P, HD], f32r)
            wq_sb = pp.tile([P, DC, DV], f32r)
            wk_sb = pp.tile([P, DC, DV], f32r)
            wv_sb = pp.tile([P, DC, DV], f32r)

            # f32r memset is rejected by the ISA, so build the ones in f32
            # and round through a DVE copy (which legally produces f32r).
            ones1 = pp.tile([P, 1], f32)
            nc.any.memset(ones1[:], 1.0)
            nc.vector.tensor_copy(ones_sb[:], ones1.broadcast_to([P, HD]))
            nc.vector.tensor_copy(
                vp_sb[:, :, :, HD:HD + 1],
                ones1.broadcast_to([P, ST, HG, 1]))

            # -------- phase A: streamed loads + projections ---------------
            nc.sync.dma_start(wq_sb[:], wqt_d[:])
            xt_tiles = [None] * QC
            xt_tiles[0] = xtp.tile([P, DC, NQ], f32r, tag="xt",
                                   name="xt_sb")
            nc.sync.dma_start(xt_tiles[0][:], xt_d[0])
            nc.sync.dma_start(wk_sb[:], wkt_d[:])
            nc.sync.dma_start(wv_sb[:], wvt_d[:])
            nc.sync.dma_start(wot_sb[:], wot_d[:])

            # attention inner segment (scores + exp + ctx for a kt range)
            # per-head 1-bank score tiles -> deeper PSUM pipelining
            do_ctx = mode not in ("sx",)
            def attn_seg(qc, t, ctA, ctB, kts):
                qsl = slice(qc * NQ, (qc + 1) * NQ)
                hA, hB = 2 * t, 2 * t + 1

                def emit_ctx(kt, pt_prev):
                    nc.tensor.matmul(
                        ctA[0:HD + 1, :], vp_sb[:, kt, hA, :],
                        pt_prev[:, 0, :], start=kt == 0, stop=kt == ST - 1)
                    nc.tensor.matmul(
                        ctB[0:HD + 1, :], vp_sb[:, kt, hB, :],
                        pt_prev[:, 1, :], start=kt == 0, stop=kt == ST - 1)

                prev = None
                for kt in kts:
                    ksl = slice(kt * P, (kt + 1) * P)
                    st_ps = stp.tile([P, 2, NQ], f32, tag="st",
                                     name="st_ps")
                    pt_sb = ptp.tile([P, 2, NQ], f32r, tag="pt",
                                     name="pt_sb")
                    nc.tensor.matmul(
                        st_ps[:, 0, :], kt_sb[0:HD, t, ksl],
                        qt_sb[0:HD, t, qsl], tile_position=(0, 0))
                    nc.tensor.matmul(
                        st_ps[:, 1, :], kt_sb[HD:P, t, ksl],
                        qt_sb[HD:P, t, qsl], tile_position=(HD, 0))
                    if mode == "noexp":
                        nc.vector.tensor_copy(pt_sb[:], st_ps[:])
                    else:
                        nc.scalar.activation(
                            pt_sb[:], st_ps[:], EXP, scale=0.125)
                    if do_ctx and prev is not None:
                        emit_ctx(prev[0], prev[1])
                    prev = (kt, pt_sb)
                if do_ctx and prev is not None:
                    emit_ctx(prev[0], prev[1])

            def norm_pair(qc, t, ctA, ctB):
                qsl = slice(qc * NQ, (qc + 1) * NQ)
                for ct, h in ((ctA, 2 * t), (ctB, 2 * t + 1)):
                    ctu = wkp.tile([P, NQ], f32, tag="ctu", name="ctu")
                    nc.vector.tensor_copy(
                        ctu[0:HD + 1, :], ct[0:HD + 1, :])
                    rt = wkp.tile([P, NQ], f32r, tag="rt", name="rt")
                    with nc.allow_low_precision(
                            reason="softmax denom recip to f32r"):
                        nc.vector.reciprocal(
                            rt[HD:HD + 1, :], ctu[HD:HD + 1, :])
                    bc = ps2.tile([P, NQ], f32, tag="a", name="bc")
                    nc.tensor.matmul(
                        bc[0:HD, :], ones_sb[HD:HD + 1, :],
                        rt[HD:HD + 1, :], tile_position=(HD, 0))
                    if h % 2 == 0:
                        nc.vector.tensor_mul(
                            ctn_sb[0:HD, t, qsl], ctu[0:HD, :], bc[0:HD, :])
                    else:
                        tmp = wkp.tile([P, NQ], f32r, tag="tmp", name="tmp")
                        nc.vector.tensor_mul(
                            tmp[0:HD, :], ctu[0:HD, :], bc[0:HD, :])
                        nc.sync.dma_start(
                            ctn_sb[HD:P, t, qsl], tmp[0:HD, :])

            ct0A = ctp.tile([P, NQ], f32, tag="ct", name="ct0A")
            ct0B = ctp.tile([P, NQ], f32, tag="ct", name="ct0B")
            for sc in range(QC):
                ssl = slice(sc * NQ, (sc + 1) * NQ)
                if xt_tiles[sc] is None:
                    xt_tiles[sc] = xtp.tile([P, DC, NQ], f32r, tag="xt",
                                            name="xt_sb")
                    nc.sync.dma_start(xt_tiles[sc][:], xt_d[sc])
                xt_sb = xt_tiles[sc]
                if mode == "dma":
                    for si in range(4):
                        sti = sc * 4 + si
                        nc.sync.dma_start(
                            out_d[sti * P:(sti + 1) * P, :],
                            xt_sb[:, 0:2, :].bitcast(f32))
                    continue
                for t in range(2):
                    for w_sb, dst in ((wq_sb, qt_sb), (wk_sb, kt_sb)):
                        ps = ps2.tile([P, NQ], f32, tag="a", name="ps")
                        for dc in range(DC):
                            nc.tensor.matmul(
                                ps[:],
                                w_sb[:, dc, t * P:(t + 1) * P],
                                xt_sb[:, dc, :],
                                start=dc == 0, stop=dc == DC - 1)
                        nc.vector.tensor_copy(dst[:, t, ssl], ps[:])
                for si in range(4):
                    sti = sc * 4 + si
                    ps = ps2.tile([P, NQ], f32, tag="a", name="ps")
                    for dc in range(DC):
                        nc.tensor.matmul(
                            ps[:, :DV],
                            xt_sb[:, dc, si * P:(si + 1) * P],
                            wv_sb[:, dc, :],
                            start=dc == 0, stop=dc == DC - 1)
                    for h in range(HG):
                        nc.vector.tensor_copy(
                            vp_sb[:, sti, h, 0:HD],
                            ps[:, h * HD:(h + 1) * HD])
                if mode in ("full", "noout", "noexp", "sx"):
                    attn_seg(0, 0, ct0A, ct0B,
                             range(sc * 4, sc * 4 + 4))
            if mode == "proj":
                for sti in range(ST):
                    nc.sync.dma_start(
                        out_d[sti * P:(sti + 1) * P, :],
                        qt_sb[:, 0, 0:D].bitcast(f32))
            # -------- phase B: attention + out-projection -----------------
            if mode in ("full", "noout", "noexp", "sx"):
                for qc in range(QC):
                    for t in range(2):
                        if qc == 0 and t == 0:
                            # already computed interleaved with projections
                            if mode != "sx":
                                norm_pair(0, 0, ct0A, ct0B)
                            continue
                        ctA = ctp.tile([P, NQ], f32, tag="ct", name="ctA")
                        ctB = ctp.tile([P, NQ], f32, tag="ct", name="ctB")
                        attn_seg(qc, t, ctA, ctB, range(ST))
                        if mode != "sx":
                            norm_pair(qc, t, ctA, ctB)
                    # out-projection for this q-chunk's s-tiles (K=128)
                    for si in range(4):
                        sti = qc * 4 + si
                        ssl = slice(sti * P, (sti + 1) * P)
                        if mode in ("noout", "sx"):
                            junk = ctn_sb if mode == "noout" else qt_sb
                            nc.sync.dma_start(
                                out_d[ssl, :],
                                junk[:, 0, 0:D].bitcast(f32))
                            continue
                        ob = osb.tile([P, D], f32, tag="ob", name="ob")
                        for ec in (0, 1):
                            esl = slice(ec * NQ, (ec + 1) * NQ)
                            op = ps2.tile([P, NQ], f32, tag="a", name="op")
                            for dvt in (0, 1):
                                nc.tensor.matmul(
                                    op[:],
                                    ctn_sb[:, dvt, ssl],
                                    wot_sb[:, dvt, esl],
                                    start=dvt == 0, stop=dvt == 1)
                            nc.vector.tensor_copy(ob[:, esl], op[:])
                        nc.sync.dma_start(out_d[ssl, :], ob[:])

    nc.compile()
    return nc


def _get_nc():
    if "nc" not in _CACHE:
        _CACHE["nc"] = _build()
    return _CACHE["nc"]


def _pack_inputs(x, Wq, Wk, Wv, Wo):
    """Host-side pre-tiling into the exact DRAM layouts the NEFF expects."""
    x = np.asarray(x, np.float32)
    in_maps = []
    for c in range(NCORES):
        b, g = divmod(c, GROUPS)
        sl = slice(g * DV, (g + 1) * DV)
        xtb = np.ascontiguousarray(x[b].T)            # [D, S]
        xt = np.ascontiguousarray(
            xtb.reshape(DC, P, QC, NQ).transpose(2, 1, 0, 3))
        wqt = np.ascontiguousarray(                   # [P, DC, DV]
            np.asarray(Wq, np.float32)[sl, :].T
            .reshape(DC, P, DV).transpose(1, 0, 2))
        wkt = np.ascontiguousarray(
            np.asarray(Wk, np.float32)[sl, :].T
            .reshape(DC, P, DV).transpose(1, 0, 2))
        wvt = np.ascontiguousarray(
            np.asarray(Wv, np.float32)[sl, :].T
            .reshape(DC, P, DV).transpose(1, 0, 2))
        wot = np.ascontiguousarray(                   # [P, 2, D]
            np.asarray(Wo, np.float32)[:, sl].T
            .reshape(2, P, D).transpose(1, 0, 2))
        in_maps.append({"xt": xt, "wqt": wqt, "wkt": wkt,
                        "wvt": wvt, "wot": wot})
    return in_maps


def kernel(x, Wq, Wk, Wv, Wo, bo, _trace=False):
    bo = np.asarray(bo, np.float32)
    in_maps = _pack_inputs(x, Wq, Wk, Wv, Wo)
    res = run_bass_kernel_spmd(
        _get_nc(), in_maps, core_ids=list(range(NCORES)), trace=_trace)
    _CACHE["last_result"] = res
    parts = [res.results[c]["out"] for c in range(NCORES)]
    out = np.empty((B, S, D), np.float32)
    for b in range(B):
        acc = np.sum(np.stack(parts[GROUPS * b:GROUPS * (b + 1)]),
                     axis=0, dtype=np.float64)
        out[b] = (acc + bo.astype(np.float64)).astype(np.float32)
    return out



# revision 2
# speedup vs baseline: 1.3149x; 1.3149x over previous
"""Multi-head attention (B=2, S=2048, D=1024, H=16) on 8 trn2 NeuronCores.

Sharding: data-parallel over batch (2) x tensor-parallel over head-groups (4).
Core c handles batch c//4, heads [4*(c%4), 4*(c%4)+4).  Each core computes
Q/K/V projections for its 4 heads, attention (no mask - the reference's
causal mask is a no-op), and a partial out-projection against its slice of
Wo.  The 4 partial outputs per batch are summed on the host (+bias), which
replaces the all-reduce.

Key engineering (vs the f32r baseline):
  - The whole pipeline runs in fp16 (10-bit mantissa, same class as the
    f32r/TF32 PE mode, fp32 PSUM accumulation): halves DMA bytes, halves
    SBUF footprint, and lets DVE PSUM->SBUF copies run in 2x packed mode.
  - exp() of the score matrix (16.8M elems/core) is split between ScalarE
    (exact ACT exp) and VectorE (Schraudolph bit-trick exp: one
    tensor_scalar computing int16(A*s+B), bitcast to fp16; ~3% max rel
    err on those tiles), so neither elementwise engine gates the PE.
  - Scores are computed transposed (ST[k,q]) with two heads row-packed in
    the PE array (K=64 each, tile_position rows 0/64); the softmax
    denominator comes from a ones-column appended to V (lhsT = [V_h | 1]):
    row 64 of the ctx accumulation is the denominator.
  - Normalization is software-pipelined one pair behind attention, and the
    out-projection one q-chunk behind that, so the PE instruction queue
    never waits on the DVE normalization chain.
"""

import numpy as np

import concourse.mybir as mybir
from concourse import bacc
from concourse.tile import TileContext
from concourse.bass_utils import run_bass_kernel_spmd

# problem constants (hardcoded; kernel.py must be self-contained)
B, S, D, H, HD = 2, 2048, 1024, 16, 64
GROUPS = 4                 # head-groups (tensor-parallel)
HG = H // GROUPS           # heads per core = 4
DV = HG * HD               # per-core qkv width = 256
P = 128
DC = D // P                # 8 contraction chunks
ST = S // P                # 16 s/k tiles
NQ = 512                   # moving free dim / q-chunk
QC = S // NQ               # 4 q-chunks
NCORES = 8

f32 = mybir.dt.float32
f16 = mybir.dt.float16
i16 = mybir.dt.int16
EXP = mybir.ActivationFunctionType.Exp
MUL = mybir.AluOpType.mult
ADD = mybir.AluOpType.add

# Schraudolph fp16 exp of (s/8):  i16 = round(A16*s + B16); bitcast fp16.
# A16 = (2^10/ln2)/8, B16 = 15*2^10 - C with C tuned for minimax rel err.
A16 = (2.0 ** 10 / np.log(2.0)) / 8.0
B16 = 15.0 * 2 ** 10 - 44.75
# k-tiles whose exp runs on the DVE (by kt % 8); rest go to ScalarE.
DVE_KTS = (2, 5, 7)

_CACHE = {}


def _build(reps=1, mode="full"):
    """Build the per-core program. reps>1 repeats the whole computation
    inside a hardware loop; mode in ("full", "noout", "sx", "noexp",
    "proj", "dma") ablates phases - only used for timing attribution."""
    nc = bacc.Bacc(None, target_bir_lowering=False, debug=False)

    # All inputs host-packed to match SBUF layouts (one contiguous DMA each)
    xt_d = nc.dram_tensor("xt", [QC, P, DC, NQ], f16, kind="ExternalInput")
    wqt_d = nc.dram_tensor("wqt", [P, DC, DV], f16, kind="ExternalInput")
    wkt_d = nc.dram_tensor("wkt", [P, DC, DV], f16, kind="ExternalInput")
    wvt_d = nc.dram_tensor("wvt", [P, DC, DV], f16, kind="ExternalInput")
    wot_d = nc.dram_tensor("wot", [P, 2, D], f16, kind="ExternalInput")
    out_d = nc.dram_tensor("out", [S, D], f16, kind="ExternalOutput")

    pairs = [(qc, t) for qc in range(QC) for t in range(2)]

    from contextlib import ExitStack
    with TileContext(nc) as tc, ExitStack() as rep_ctx:
        if reps > 1:
            rep_ctx.enter_context(tc.For_i(0, reps, 1))
        with (
            tc.tile_pool(name="persist", bufs=1) as pp,
            tc.tile_pool(name="xtp", bufs=2) as xtp,
            tc.tile_pool(name="pt", bufs=8) as ptp,
            tc.tile_pool(name="osb", bufs=3) as osb,
            tc.tile_pool(name="wkp", bufs=2) as wkp,
            tc.tile_pool(name="ps2", bufs=2, space="PSUM") as ps2,
            tc.tile_pool(name="stp", bufs=2, space="PSUM") as stp,
            tc.tile_pool(name="ctp", bufs=2, space="PSUM") as ctp,
        ):
            # QT/KT tile t holds heads 2t (partitions 0:64) and 2t+1 (64:128)
            qt_sb = pp.tile([P, 2, S], f16)
            kt_sb = pp.tile([P, 2, S], f16)
            # V per (k-tile, head) with a ones column appended ([V | 1])
            vp_sb = pp.tile([P, ST, HG, HD + 1], f16)
            # normalized ctx^T paired like QT/KT: tile t = heads 2t, 2t+1
            ctn_sb = pp.tile([P, 2, S], f16)
            wot_sb = pp.tile([P, 2, D], f16)
            ones_sb = pp.tile([P, HD], f16)
            wq_sb = pp.tile([P, DC, DV], f16)
            wk_sb = pp.tile([P, DC, DV], f16)
            wv_sb = pp.tile([P, DC, DV], f16)

            nc.any.memset(ones_sb[:], 1.0)
            nc.any.memset(vp_sb[:, :, :, HD:HD + 1], 1.0)

            # -------- phase A: streamed loads + projections ---------------
            nc.sync.dma_start(wq_sb[:], wqt_d[:])
            xt_tiles = [None] * QC
            xt_tiles[0] = xtp.tile([P, DC, NQ], f16, tag="xt", name="xt_sb")
            nc.sync.dma_start(xt_tiles[0][:], xt_d[0])
            nc.sync.dma_start(wk_sb[:], wkt_d[:])
            nc.sync.dma_start(wv_sb[:], wvt_d[:])
            nc.sync.dma_start(wot_sb[:], wot_d[:])

            def emit_exp(kt, st_ps, pt_sb):
                if mode == "noexp":
                    with nc.allow_low_precision(reason="ablation copy"):
                        nc.vector.tensor_copy(pt_sb[:], st_ps[:])
                elif (kt % 8) in DVE_KTS:
                    with nc.allow_low_precision(
                            reason="schraudolph fp16 exp"):
                        nc.vector.tensor_scalar(
                            pt_sb[:].bitcast(i16), st_ps[:],
                            A16, B16, MUL, ADD)
                else:
                    with nc.allow_low_precision(reason="fp16 softmax probs"):
                        nc.scalar.activation(
                            pt_sb[:], st_ps[:], EXP, scale=0.125)

            # attention inner segment (scores + exp + ctx for a kt range);
            # `prev` delays each ctx pair one kt so exp(kt) overlaps ctx(kt-1)
            do_ctx = mode not in ("sx",)

            def attn_seg(qc, t, ctA, ctB, kts, prev):
                qsl = slice(qc * NQ, (qc + 1) * NQ)
                hA, hB = 2 * t, 2 * t + 1

                def emit_ctx(kt, pt_prev):
                    nc.tensor.matmul(
                        ctA[0:HD + 1, :], vp_sb[:, kt, hA, :],
                        pt_prev[:, 0, :], start=kt == 0, stop=kt == ST - 1)
                    nc.tensor.matmul(
                        ctB[0:HD + 1, :], vp_sb[:, kt, hB, :],
                        pt_prev[:, 1, :], start=kt == 0, stop=kt == ST - 1)

                for kt in kts:
                    ksl = slice(kt * P, (kt + 1) * P)
                    st_ps = stp.tile([P, 2, NQ], f32, tag="st", name="st_ps")
                    pt_sb = ptp.tile([P, 2, NQ], f16, tag="pt", name="pt_sb")
                    nc.tensor.matmul(
                        st_ps[:, 0, :], kt_sb[0:HD, t, ksl],
                        qt_sb[0:HD, t, qsl], tile_position=(0, 0))
                    nc.tensor.matmul(
                        st_ps[:, 1, :], kt_sb[HD:P, t, ksl],
                        qt_sb[HD:P, t, qsl], tile_position=(HD, 0))
                    emit_exp(kt, st_ps, pt_sb)
                    if do_ctx and prev is not None:
                        emit_ctx(prev[0], prev[1])
                    prev = (kt, pt_sb)
                return prev

            def flush_ctx(qc, t, ctA, ctB, prev):
                if do_ctx and prev is not None:
                    hA, hB = 2 * t, 2 * t + 1
                    kt, pt_prev = prev
                    nc.tensor.matmul(
                        ctA[0:HD + 1, :], vp_sb[:, kt, hA, :],
                        pt_prev[:, 0, :], start=kt == 0, stop=kt == ST - 1)
                    nc.tensor.matmul(
                        ctB[0:HD + 1, :], vp_sb[:, kt, hB, :],
                        pt_prev[:, 1, :], start=kt == 0, stop=kt == ST - 1)

            # -- normalization, software-pipelined in three steps ----------
            # step 1 (right after a pair's last ctx): DVE drains ctx PSUM to
            #   SBUF (fp16 2x mode) and computes the denominator reciprocal;
            # step 2 (early in the NEXT pair): PE broadcasts the reciprocal
            #   across 64 partitions with a K=1 matmul;
            # step 3: DVE multiplies, writing normalized ctx^T (odd heads
            #   staged and DMA-shifted to partitions 64:128).
            def norm_drain(p, st8):
                ctA, ctB = st8["ct"][p]
                ctus, rts = [], []
                for ct in (ctA, ctB):
                    ctu = wkp.tile([P, NQ], f16, tag="ctu", name="ctu")
                    rt = wkp.tile([P, NQ], f16, tag="rt", name="rt")
                    with nc.allow_low_precision(reason="fp16 ctx drain"):
                        nc.vector.tensor_copy(
                            ctu[0:HD + 1, :], ct[0:HD + 1, :])
                        nc.vector.reciprocal(
                            rt[HD:HD + 1, :], ctu[HD:HD + 1, :])
                    ctus.append(ctu)
                    rts.append(rt)
                st8["norm"][p] = (ctus, rts)

            def norm_bc(p, st8):
                bcs = []
                for rt in st8["norm"][p][1]:
                    bc = ps2.tile([P, NQ], f32, tag="a", name="bc")
                    nc.tensor.matmul(
                        bc[0:HD, :], ones_sb[HD:HD + 1, :],
                        rt[HD:HD + 1, :], tile_position=(HD, 0))
                    bcs.append(bc)
                st8["bc"][p] = bcs

            def norm_mul(p, st8):
                qc, t = pairs[p]
                qsl = slice(qc * NQ, (qc + 1) * NQ)
                ctus, _ = st8["norm"][p]
                bcs = st8["bc"][p]
                with nc.allow_low_precision(reason="fp16 normalized ctx"):
                    nc.vector.tensor_mul(
                        ctn_sb[0:HD, t, qsl], ctus[0][0:HD, :],
                        bcs[0][0:HD, :])
                    tmp = wkp.tile([P, NQ], f16, tag="tmp", name="tmp")
                    nc.vector.tensor_mul(
                        tmp[0:HD, :], ctus[1][0:HD, :], bcs[1][0:HD, :])
                nc.sync.dma_start(ctn_sb[HD:P, t, qsl], tmp[0:HD, :])

            def out_proj(qc):
                for si in range(4):
                    sti = qc * 4 + si
                    ssl = slice(sti * P, (sti + 1) * P)
                    if mode in ("noout", "sx"):
                        nc.sync.dma_start(
                            out_d[ssl, :], ctn_sb[:, 0, 0:D])
                        continue
                    ob = osb.tile([P, D], f16, tag="ob", name="ob")
                    for ec in (0, 1):
                        esl = slice(ec * NQ, (ec + 1) * NQ)
                        op = ps2.tile([P, NQ], f32, tag="a", name="op")
                        for dvt in (0, 1):
                            nc.tensor.matmul(
                                op[:],
                                ctn_sb[:, dvt, ssl],
                                wot_sb[:, dvt, esl],
                                start=dvt == 0, stop=dvt == 1)
                        with nc.allow_low_precision(
                                reason="fp16 partial output"):
                            nc.vector.tensor_copy(ob[:, esl], op[:])
                    nc.sync.dma_start(out_d[ssl, :], ob[:])

            # -------- phase A body ----------------------------------------
            st8 = {"ct": {}, "norm": {}, "bc": {}}
            do_attn = mode in ("full", "noout", "noexp", "sx")
            if do_attn:
                ct0A = ctp.tile([P, NQ], f32, tag="ct", name="ct0A")
                ct0B = ctp.tile([P, NQ], f32, tag="ct", name="ct0B")
                st8["ct"][0] = (ct0A, ct0B)
            prev0 = None
            for sc in range(QC):
                ssl = slice(sc * NQ, (sc + 1) * NQ)
                if xt_tiles[sc] is None:
                    xt_tiles[sc] = xtp.tile([P, DC, NQ], f16, tag="xt",
                                            name="xt_sb")
                    nc.sync.dma_start(xt_tiles[sc][:], xt_d[sc])
                xt_sb = xt_tiles[sc]
                if mode == "dma":
                    for si in range(4):
                        sti = sc * 4 + si
                        nc.sync.dma_start(
                            out_d[sti * P:(sti + 1) * P, :],
                            xt_sb[:, 0:2, :])
                    continue
                for t in range(2):
                    for w_sb, dst in ((wq_sb, qt_sb), (wk_sb, kt_sb)):
                        ps = ps2.tile([P, NQ], f32, tag="a", name="ps")
                        for dc in range(DC):
                            nc.tensor.matmul(
                                ps[:],
                                w_sb[:, dc, t * P:(t + 1) * P],
                                xt_sb[:, dc, :],
                                start=dc == 0, stop=dc == DC - 1)
                        # ScalarE drains the QK projections (DVE does V) -
                        # ScalarE is otherwise idle during phase A
                        with nc.allow_low_precision(reason="fp16 q/k"):
                            nc.scalar.copy(dst[:, t, ssl], ps[:])
                for si in range(4):
                    sti = sc * 4 + si
                    ps = ps2.tile([P, NQ], f32, tag="a", name="ps")
                    for dc in range(DC):
                        nc.tensor.matmul(
                            ps[:, :DV],
                            xt_sb[:, dc, si * P:(si + 1) * P],
                            wv_sb[:, dc, :],
                            start=dc == 0, stop=dc == DC - 1)
                    with nc.allow_low_precision(reason="fp16 v"):
                        nc.vector.tensor_copy(
                            vp_sb[:, sti, :, 0:HD],
                            ps[:, 0:DV].rearrange(
                                "p (h d) -> p h d", h=HG))
                if do_attn:
                    prev0 = attn_seg(0, 0, ct0A, ct0B,
                                     range(sc * 4, sc * 4 + 4), prev0)
            if mode == "proj":
                for sti in range(ST):
                    nc.sync.dma_start(
                        out_d[sti * P:(sti + 1) * P, :],
                        qt_sb[:, 0, 0:D])
            # -------- phase B: attention + norm + out-projection ----------
            if do_attn:
                flush_ctx(0, 0, ct0A, ct0B, prev0)
                if mode != "sx":
                    norm_drain(0, st8)
                for p in range(1, 8):
                    qc, t = pairs[p]
                    ctA = ctp.tile([P, NQ], f32, tag="ct", name="ctA")
                    ctB = ctp.tile([P, NQ], f32, tag="ct", name="ctB")
                    st8["ct"][p] = (ctA, ctB)
                    prev = attn_seg(qc, t, ctA, ctB, range(0, 2), None)
                    if mode != "sx":
                        norm_bc(p - 1, st8)
                    if p in (3, 5, 7):
                        out_proj((p - 3) // 2)
                    prev = attn_seg(qc, t, ctA, ctB, range(2, ST), prev)
                    flush_ctx(qc, t, ctA, ctB, prev)
                    if mode != "sx":
                        norm_mul(p - 1, st8)
                        norm_drain(p, st8)
                if mode != "sx":
                    norm_bc(7, st8)
                    norm_mul(7, st8)
                out_proj(3)

    nc.compile()
    return nc


def _get_nc():
    if "nc" not in _CACHE:
        _CACHE["nc"] = _build()
    return _CACHE["nc"]


def _pack_inputs(x, Wq, Wk, Wv, Wo):
    """Host-side pre-tiling into the exact DRAM layouts the NEFF expects."""
    x = np.asarray(x, np.float32)
    in_maps = []
    for c in range(NCORES):
        b, g = divmod(c, GROUPS)
        sl = slice(g * DV, (g + 1) * DV)
        xtb = np.ascontiguousarray(x[b].T)            # [D, S]
        xt = np.ascontiguousarray(
            xtb.reshape(DC, P, QC, NQ).transpose(2, 1, 0, 3)
        ).astype(np.float16)
        wqt = np.ascontiguousarray(                   # [P, DC, DV]
            np.asarray(Wq, np.float32)[sl, :].T
            .reshape(DC, P, DV).transpose(1, 0, 2)).astype(np.float16)
        wkt = np.ascontiguousarray(
            np.asarray(Wk, np.float32)[sl, :].T
            .reshape(DC, P, DV).transpose(1, 0, 2)).astype(np.float16)
        wvt = np.ascontiguousarray(
            np.asarray(Wv, np.float32)[sl, :].T
            .reshape(DC, P, DV).transpose(1, 0, 2)).astype(np.float16)
        wot = np.ascontiguousarray(                   # [P, 2, D]
            np.asarray(Wo, np.float32)[:, sl].T
            .reshape(2, P, D).transpose(1, 0, 2)).astype(np.float16)
        in_maps.append({"xt": xt, "wqt": wqt, "wkt": wkt,
                        "wvt": wvt, "wot": wot})
    return in_maps


def kernel(x, Wq, Wk, Wv, Wo, bo, _trace=False):
    bo = np.asarray(bo, np.float32)
    in_maps = _pack_inputs(x, Wq, Wk, Wv, Wo)
    res = run_bass_kernel_spmd(
        _get_nc(), in_maps, core_ids=list(range(NCORES)), trace=_trace)
    _CACHE["last_result"] = res
    parts = [res.results[c]["out"] for c in range(NCORES)]
    out = np.empty((B, S, D), np.float32)
    for b in range(B):
        acc = np.sum(
            np.stack([p.astype(np.float32)
                      for p in parts[GROUPS * b:GROUPS * (b + 1)]]),
            axis=0, dtype=np.float32)
        out[b] = acc + bo
    return out
